# revision 6
# baseline (speedup 1.0000x reference)
"""Bass/Trainium2 kernel for nn_HNO_37065567764989 (self-contained).

Strategy (8 NeuronCores, SPMD):
- The 128x524288 branch matvec b = Wb@a is column-sharded 8 ways. Each core
  streams its 32MB shard as bf16 hi/lo pairs (exact-ish split, ~4e-6 rel) with
  the [a_hi|a_lo] pair as a K=128/M=2 stationary and [W_hi|W_lo] as one N=256
  moving operand, accumulating all four cross terms in PSUM. A 512B AllReduce
  combines the per-core partials.
- The Nx=32768 evaluation points are sharded 8 ways (4096/core). The trunk
  chains (b-independent) overlap the Wb DMA; the EnergyNet phase runs after
  the AllReduce, chunk-stacked two 64-wide halves into 128 partitions.
- Output is -d/dx F_y + d^2/dx^2 F_z computed via forward derivative chains;
  all constant factors fold into matmul stationaries, sums fold into PSUM
  accumulation.
"""
import sys

for _p in ("/opt/trn_rl_repo",):
    if _p not in sys.path:
        sys.path.insert(0, _p)

import numpy as np
import ml_dtypes

MP1, NX, P, HT, HE = 524288, 32768, 128, 128, 64
NCORES = 8
KSH = MP1 // NCORES       # 65536
NCHUNK = 32
JT = KSH // NCHUNK // 128  # 16 k-tiles per chunk
NPTS = NX // NCORES        # 4096 points per core
FD = 512
NTRUNK = NPTS // FD        # 8 trunk tiles
NEN = NPTS // 2 // FD      # 4 energy tiles (chunk-stacked)

_CACHE = {}


def _build(mmdt_name):
    import concourse.bacc as bacc
    import concourse.mybir as mybir
    from concourse import tile

    f32 = mybir.dt.float32
    bf16 = mybir.dt.bfloat16
    MMDT = getattr(mybir.dt, mmdt_name)
    AF = mybir.ActivationFunctionType
    ALU = mybir.AluOpType

    nc = bacc.Bacc("TRN2", target_bir_lowering=False, debug=False,
                   num_devices=NCORES)

    w_d = nc.dram_tensor("w", [NCHUNK, 2, 128, JT * 128], bf16, kind="ExternalInput")
    a_d = nc.dram_tensor("a2", [128, KSH // 128, 2], bf16, kind="ExternalInput")
    x2_d = nc.dram_tensor("x2", [2, NPTS], MMDT, kind="ExternalInput")
    sm = {}
    for name, shape, dt in [
        ("w10_2", [2, 128], MMDT), ("c1b", [128, 1], f32), ("bt2b", [128, 1], f32),
        ("wt2t", [128, 128], MMDT), ("w2at", [128, 128], MMDT),
        ("w2bt", [128, 128], MMDT), ("w2ct", [128, 128], MMDT),
        ("wt3", [128, 128], f32), ("bt3c", [128, 1], f32),
        ("e0", [128, 128], MMDT), ("e0m2", [128, 128], MMDT),
        ("eq", [128, 128], MMDT), ("eqm2", [128, 128], MMDT),
        ("eq6", [128, 128], MMDT), ("ep", [128, 128], MMDT),
        ("epm2", [128, 128], MMDT),
        ("be1b2", [128, 1], f32), ("be2b2", [128, 1], f32),
        ("lmat", [4, 768], MMDT), ("v6", [128, 12], MMDT),
    ]:
        sm[name] = nc.dram_tensor(name, shape, dt, kind="ExternalInput")
    out_d = nc.dram_tensor("out", [2, NPTS // 2], f32, kind="ExternalOutput")
    cc_in = nc.dram_tensor("cc_in", [128, 1], f32)
    cc_out = nc.dram_tensor("cc_out", [128, 1], f32, addr_space="Shared")

    def TT(eng, out, a, b, op=ALU.mult):
        eng.tensor_tensor(out, a, b, op)

    with tile.TileContext(nc) as tc:
        with (
            tc.tile_pool(name="smp", bufs=1) as smp,
            tc.tile_pool(name="persist", bufs=1) as persist,
            tc.tile_pool(name="wpool", bufs=2) as wpool,
            tc.tile_pool(name="scr", bufs=1) as scr,
            tc.tile_pool(name="u8p", bufs=1) as u8p,
        ):
            # small weights to SBUF
            smt = {}
            for name, h in sm.items():
                t = smp.tile(list(h.shape), h.dtype, name=f"sb_{name}")
                nc.sync.dma_start(t[:], h.ap())
                smt[name] = t
            a2 = smp.tile([128, KSH // 128, 2], bf16, name="a2t")
            nc.sync.dma_start(a2[:], a_d.ap())
            ones2 = smp.tile([2, 1], f32, name="ones2")
            nc.vector.memset(ones2[:], 1.0)

            persist_t = {}
            with (
                tc.tile_pool(name="mvps", bufs=1, space="PSUM") as mvps,
                tc.tile_pool(name="tkps", bufs=1, space="PSUM") as tkps,
            ):
                # ---- matvec: stream Wb shard ----
                b2_ps = mvps.tile([2, 256], f32, name="b2ps")
                for i in range(NCHUNK):
                    wch = wpool.tile([128, 2, JT * 128], bf16, tag="wch", name="wch")
                    nc.sync.dma_start(wch[:], w_d.ap()[i].rearrange("s kp x -> kp s x"))
                    for j in range(JT):
                        jj = i * JT + j
                        nc.tensor.matmul(
                            b2_ps[:], a2[:, jj, :],
                            wch[:, :, j * 128:(j + 1) * 128],
                            start=(jj == 0), stop=(jj == NCHUNK * JT - 1),
                        )
                b2_sb = smp.tile([2, 256], f32, name="b2sb")
                nc.scalar.copy(b2_sb[:], b2_ps[:])
                bcol_ps = mvps.tile([128, 1], f32, name="bcolps")
                nc.tensor.matmul(bcol_ps[:], b2_sb[:, 0:128], ones2[:], start=True, stop=False)
                nc.tensor.matmul(bcol_ps[:], b2_sb[:, 128:256], ones2[:], start=False, stop=True)
                b_loc = smp.tile([128, 1], f32, name="bloc")
                nc.scalar.copy(b_loc[:], bcol_ps[:])
                nc.sync.dma_start(cc_in.ap(), b_loc[:])
                nc.gpsimd.collective_compute(
                    "AllReduce", ALU.add,
                    replica_groups=[list(range(NCORES))],
                    ins=[cc_in.ap()], outs=[cc_out.ap()],
                )
                b_ar = smp.tile([128, 1], f32, name="bar")
                nc.sync.dma_start(b_ar[:], cc_out.ap())

                # ---- trunk (b-independent), overlaps the matvec DMA ----
                for f in range(NTRUNK):
                    cs = slice(f * FD, (f + 1) * FD)
                    xt = scr.tile([2, FD], MMDT, tag="xt", name="xt", bufs=2)
                    nc.sync.dma_start(xt[:], x2_d.ap()[:, cs])
                    z1 = tkps.tile([128, FD], f32, tag="z1", name="z1")
                    nc.tensor.matmul(z1[:], smt["w10_2"][:], xt[:], start=True, stop=True)
                    t1 = scr.tile([128, FD], MMDT, tag="t1", name="t1")
                    nc.scalar.activation(t1[:], z1[:], AF.Tanh, bias=smt["c1b"][:])
                    t1f = t1[:].bitcast(f32)
                    s1 = scr.tile([128, FD], f32, tag="s1", name="s1")
                    nc.scalar.square(s1[:], t1f)
                    tp1 = scr.tile([128, FD], MMDT, tag="tp1", name="tp1")
                    nc.vector.tensor_scalar(tp1[:], s1[:], -1.0, 1.0, ALU.mult, ALU.add)
                    tp1f = tp1[:].bitcast(f32)
                    g2m = scr.tile([128, FD], MMDT, tag="g2m", name="g2m")
                    TT(nc.vector, g2m[:], t1f, tp1f)
                    g3m = scr.tile([128, FD], MMDT, tag="g3m", name="g3m")
                    nc.vector.scalar_tensor_tensor(
                        g3m[:], s1[:], 1.0 / 3.0, tp1f, ALU.subtract, ALU.mult)
                    z2 = tkps.tile([128, FD], f32, tag="z2", name="z2")
                    nc.tensor.matmul(z2[:], smt["wt2t"][:], t1[:], start=True, stop=True)
                    A = tkps.tile([128, FD], f32, tag="A", name="A")
                    nc.tensor.matmul(A[:], smt["w2at"][:], tp1[:], start=True, stop=True)
                    B = tkps.tile([128, FD], f32, tag="B", name="B")
                    nc.tensor.matmul(B[:], smt["w2bt"][:], g2m[:], start=True, stop=True)
                    C = tkps.tile([128, FD], f32, tag="C", name="C")
                    nc.tensor.matmul(C[:], smt["w2ct"][:], g3m[:], start=True, stop=True)

                    t2 = persist.tile([128, FD], MMDT, tag=f"t2_{f}", name=f"t2_{f}")
                    nc.scalar.activation(t2[:], z2[:], AF.Tanh, bias=smt["bt2b"][:])
                    t2f = t2[:].bitcast(f32)
                    s2 = scr.tile([128, FD], f32, tag="s2", name="s2")
                    nc.scalar.square(s2[:], t2f)
                    tp2 = scr.tile([128, FD], f32, tag="tp2", name="tp2")
                    nc.vector.tensor_scalar(tp2[:], s2[:], -1.0, 1.0, ALU.mult, ALU.add)
                    A2 = scr.tile([128, FD], f32, tag="A2", name="A2")
                    nc.scalar.square(A2[:], A[:])
                    P1 = persist.tile([128, FD], MMDT, tag=f"P1_{f}", name=f"P1_{f}")
                    TT(nc.vector, P1[:], tp2[:], A[:])
                    M4 = scr.tile([128, FD], f32, tag="M4", name="M4")
                    TT(nc.vector, M4[:], tp2[:], A2[:])
                    M5 = scr.tile([128, FD], f32, tag="M5", name="M5")
                    TT(nc.gpsimd, M5[:], t2f, M4[:])
                    M6 = scr.tile([128, FD], f32, tag="M6", name="M6")
                    TT(nc.vector, M6[:], tp2[:], B[:])
                    # uxxM = -2*M5 + M6
                    uxxM = persist.tile([128, FD], MMDT, tag=f"ux2_{f}", name=f"ux2_{f}")
                    nc.vector.scalar_tensor_tensor(
                        uxxM[:], M5[:], -2.0, M6[:], ALU.mult, ALU.add)
                    A3 = scr.tile([128, FD], f32, tag="A3", name="A3")
                    TT(nc.vector, A3[:], A2[:], A[:])
                    V = scr.tile([128, FD], f32, tag="V", name="V")
                    nc.vector.scalar_tensor_tensor(
                        V[:], s2[:], 1.0 / 3.0, tp2[:], ALU.subtract, ALU.mult)
                    M1 = scr.tile([128, FD], f32, tag="M1", name="M1")
                    TT(nc.gpsimd, M1[:], V[:], A3[:])
                    W1 = scr.tile([128, FD], f32, tag="W1", name="W1")
                    TT(nc.vector, W1[:], P1[:].bitcast(f32), B[:])
                    M2 = scr.tile([128, FD], f32, tag="M2", name="M2")
                    TT(nc.gpsimd, M2[:], t2f, W1[:])
                    M3 = scr.tile([128, FD], f32, tag="M3", name="M3")
                    TT(nc.vector, M3[:], tp2[:], C[:])
                    # uxxxM = 6*(M1 - M2) + M3
                    D1 = scr.tile([128, FD], f32, tag="D1", name="D1")
                    TT(nc.vector, D1[:], M1[:], M2[:], ALU.subtract)
                    uxxxM = persist.tile([128, FD], MMDT, tag=f"ux3_{f}", name=f"ux3_{f}")
                    nc.vector.scalar_tensor_tensor(
                        uxxxM[:], D1[:], 6.0, M3[:], ALU.mult, ALU.add)
                    persist_t[f] = (t2, P1, uxxM, uxxxM)

            # ---- post-AllReduce: c, d, extracts ----
            with tc.tile_pool(name="exps", bufs=1, space="PSUM") as exps:
                c_ps = exps.tile([128, 1], f32, name="cps")
                nc.tensor.matmul(c_ps[:], smt["wt3"][:], b_ar[:], start=True, stop=True)
                c_sb = smp.tile([128, 1], MMDT, name="csb")
                nc.scalar.copy(c_sb[:], c_ps[:])
                d_ps = exps.tile([1, 1], f32, name="dps")
                nc.tensor.matmul(d_ps[:], smt["bt3c"][:], b_ar[:], start=True, stop=True)
                d_sb = smp.tile([1, 1], f32, name="dsb")
                nc.scalar.copy(d_sb[:], d_ps[:])
                u8_tiles = {}
                for f in range(NTRUNK):
                    t2, P1, uxxM, uxxxM = persist_t[f]
                    half = u8p.tile([4, FD], MMDT, tag=f"u8_{f}", name=f"u8_{f}")
                    u8_tiles[f] = half
                    for r, (mov, bias) in enumerate([
                        (t2, d_sb), (P1, None), (uxxM, None), (uxxxM, None),
                    ]):
                        ue = exps.tile([1, FD], f32, tag=f"ue{r}", name=f"ue{r}")
                        nc.tensor.matmul(ue[:], c_sb[:], mov[:], start=True, stop=True)
                        ur = scr.tile([1, FD], MMDT, tag=f"ur{r}", name=f"ur{r}")
                        if bias is not None:
                            nc.scalar.activation(ur[:], ue[:], AF.Identity, bias=bias[:])
                        else:
                            nc.scalar.copy(ur[:], ue[:])
                        nc.sync.dma_start(half[r:r + 1, :], ur[:])

            # ---- energy phase (chunk-stacked, after extracts) ----
            with tc.tile_pool(name="enps", bufs=1, space="PSUM") as enps:
                L = smt["lmat"]
                for e in range(NEN):
                    hA = u8_tiles[e]
                    hB = u8_tiles[e + NEN]
                    z1e = enps.tile([128, FD], f32, tag="pA", name="z1e")
                    z1p = enps.tile([128, FD], f32, tag="pB", name="z1p")
                    z1pp = enps.tile([128, FD], f32, tag="pC", name="z1pp")
                    for ps_t, li in ((z1e, 0), (z1p, 1), (z1pp, 2)):
                        nc.tensor.matmul(ps_t[:], L[:, li * 256:li * 256 + 128],
                                         hA[:], start=True, stop=False)
                        nc.tensor.matmul(ps_t[:], L[:, li * 256 + 128:(li + 1) * 256],
                                         hB[:], start=False, stop=True)
                    t1e = scr.tile([128, FD], MMDT, tag="t1", name="t1e")
                    nc.scalar.activation(t1e[:], z1e[:], AF.Tanh, bias=smt["be1b2"][:])
                    t1ef = t1e[:].bitcast(f32)
                    s1e = scr.tile([128, FD], f32, tag="s1", name="s1e")
                    nc.scalar.square(s1e[:], t1ef)
                    m = scr.tile([128, FD], MMDT, tag="tp1", name="m")
                    nc.vector.tensor_scalar(m[:], s1e[:], -1.0, 1.0, ALU.mult, ALU.add)
                    mf = m[:].bitcast(f32)
                    z1p2 = scr.tile([128, FD], f32, tag="g2m", name="z1p2")
                    nc.scalar.square(z1p2[:], z1p[:])
                    N1 = scr.tile([128, FD], f32, tag="g3m", name="N1")
                    TT(nc.gpsimd, N1[:], t1ef, mf)
                    a1p = scr.tile([128, FD], MMDT, tag="s2", name="a1p")
                    TT(nc.vector, a1p[:], mf, z1p[:])
                    N2 = scr.tile([128, FD], MMDT, tag="tp2", name="N2")
                    TT(nc.gpsimd, N2[:], N1[:], z1p2[:])
                    N3 = scr.tile([128, FD], MMDT, tag="A2", name="N3")
                    TT(nc.vector, N3[:], mf, z1pp[:])
                    mpc = scr.tile([128, FD], MMDT, tag="M4", name="mpc")
                    TT(nc.vector, mpc[:], N1[:], z1p[:])
                    O1 = scr.tile([128, FD], f32, tag="M5", name="O1")
                    nc.vector.scalar_tensor_tensor(
                        O1[:], s1e[:], 1.0 / 3.0, mf, ALU.subtract, ALU.mult)
                    O2 = scr.tile([128, FD], MMDT, tag="M6", name="O2")
                    TT(nc.gpsimd, O2[:], O1[:], z1p2[:])
                    O3 = scr.tile([128, FD], MMDT, tag="A3", name="O3")
                    TT(nc.vector, O3[:], N1[:], z1pp[:])

                    z2e = enps.tile([128, FD], f32, tag="pD", name="z2e")
                    nc.tensor.matmul(z2e[:], smt["e0"][:], t1e[:], start=True, stop=True)
                    t2e = scr.tile([128, FD], f32, tag="V", name="t2e")
                    nc.scalar.activation(t2e[:], z2e[:], AF.Tanh, bias=smt["be2b2"][:])
                    z2ep = enps.tile([128, FD], f32, tag="pE", name="z2ep")
                    nc.tensor.matmul(z2ep[:], smt["e0"][:], a1p[:], start=True, stop=True)
                    z2epp = enps.tile([128, FD], f32, tag="pF", name="z2epp")
                    nc.tensor.matmul(z2epp[:], smt["e0m2"][:], N2[:], start=True, stop=False)
                    nc.tensor.matmul(z2epp[:], smt["e0"][:], N3[:], start=False, stop=True)
                    Dz = enps.tile([128, FD], f32, tag="pG", name="Dz")
                    nc.tensor.matmul(Dz[:], smt["eq"][:], m[:], start=True, stop=True)
                    Dy = enps.tile([128, FD], f32, tag="pH", name="Dy")
                    nc.tensor.matmul(Dy[:], smt["ep"][:], m[:], start=True, stop=True)
                    Dzp = enps.tile([128, FD], f32, tag="pA", name="Dzp")
                    nc.tensor.matmul(Dzp[:], smt["eqm2"][:], mpc[:], start=True, stop=True)
                    Dyp = enps.tile([128, FD], f32, tag="pB", name="Dyp")
                    nc.tensor.matmul(Dyp[:], smt["epm2"][:], mpc[:], start=True, stop=True)
                    Dzpp = enps.tile([128, FD], f32, tag="pC", name="Dzpp")
                    nc.tensor.matmul(Dzpp[:], smt["eq6"][:], O2[:], start=True, stop=False)
                    nc.tensor.matmul(Dzpp[:], smt["eqm2"][:], O3[:], start=False, stop=True)

                    s2e = scr.tile([128, FD], f32, tag="M1", name="s2e")
                    nc.scalar.square(s2e[:], t2e[:])
                    w = scr.tile([128, FD], f32, tag="AB", name="w")
                    nc.vector.tensor_scalar(w[:], s2e[:], -1.0, 1.0, ALU.mult, ALU.add)
                    z2ep2 = scr.tile([128, FD], f32, tag="W1", name="z2ep2")
                    nc.scalar.square(z2ep2[:], z2ep[:])
                    Q1 = scr.tile([128, FD], f32, tag="M2", name="Q1")
                    TT(nc.gpsimd, Q1[:], t2e[:], w[:])
                    wpc = scr.tile([128, FD], f32, tag="M3", name="wpc")
                    TT(nc.vector, wpc[:], Q1[:], z2ep[:])
                    R1 = scr.tile([128, FD], f32, tag="D1", name="R1")
                    nc.vector.scalar_tensor_tensor(
                        R1[:], s2e[:], 1.0 / 3.0, w[:], ALU.subtract, ALU.mult)
                    R2 = scr.tile([128, FD], f32, tag="ur0", name="R2")
                    TT(nc.gpsimd, R2[:], R1[:], z2ep2[:])
                    R3 = scr.tile([128, FD], f32, tag="ur1", name="R3")
                    TT(nc.vector, R3[:], Q1[:], z2epp[:])

                    vps = enps.tile([2, FD], f32, tag="pD", name="vps")
                    fsrc = [(R2, Dz), (R3, Dz), (wpc, Dzp), (wpc, Dy), (w, Dzpp), (w, Dyp)]
                    for i, (x1, x2_) in enumerate(fsrc):
                        Fi = scr.tile([128, FD], MMDT, tag=["ur2", "ur3", "fm2"][i % 3], name=f"f{i}")
                        TT(nc.vector, Fi[:], x1[:], x2_[:])
                        nc.tensor.matmul(vps[:], smt["v6"][:, 2 * i:2 * i + 2], Fi[:],
                                         start=(i == 0), stop=(i == 5))
                    ot = scr.tile([2, FD], f32, tag="ot", name="ot")
                    nc.scalar.copy(ot[:], vps[:])
                    nc.sync.dma_start(
                        out_d.ap()[:, e * FD:(e + 1) * FD], ot[:])

    nc.compile()
    return nc


def _get_nc(mmdt_name):
    if mmdt_name not in _CACHE:
        _CACHE[mmdt_name] = _build(mmdt_name)
    return _CACHE[mmdt_name]


MMDT_NAME = "float32r"


def kernel(**inputs):
    import concourse.bass_utils as bass_utils

    f = lambda k: np.asarray(inputs[k], np.float32)
    a, x, t = f("a"), f("x"), np.float32(inputs["t"])
    Wb, Wt1, bt1, Wt2, bt2 = f("Wb"), f("Wt1"), f("bt1"), f("Wt2"), f("bt2")
    Wt3, bt3, We1, be1, We2, be2, We3 = (
        f("Wt3"), f("bt3"), f("We1"), f("be1"), f("We2"), f("be2"), f("We3"))
    bb, be3 = f("bb"), f("be3")

    w1 = Wt1[:, 0]
    c1b = (Wt1[:, 1] * t + bt1)[:, None]
    p, q, v = We1[:, 0], We1[:, 1], We3[0]
    blk = lambda M: np.block([[M, np.zeros_like(M)], [np.zeros_like(M), M]])
    We2T = We2.T
    lmat = np.zeros((4, 768), np.float32)
    for li, pat in enumerate([(p, q, None, None), (None, p, q, None), (None, None, p, q)]):
        for r, vec in enumerate(pat):
            if vec is not None:
                lmat[r, li * 256:li * 256 + 64] = 0  # placeholder, set below
    # build lmat properly: cols 0-127 chunk A, 128-255 chunk B per member
    lmat = np.zeros((4, 768), np.float32)
    for li in range(3):
        rows = [(li + 0, p), (li + 1, q)]
        for r, vec in rows:
            lmat[r, li * 256:li * 256 + 64] = vec
            lmat[r, li * 256 + 128 + 64:li * 256 + 256] = vec
    v6 = np.zeros((128, 12), np.float32)
    for i, coef in enumerate([6.0, -2.0, -4.0, 2.0, 1.0, -1.0]):
        v6[0:64, 2 * i] = coef * v
        v6[64:128, 2 * i + 1] = coef * v

    smalls = {
        "w10_2": np.stack([w1, np.zeros_like(w1)]),
        "c1b": c1b, "bt2b": bt2[:, None],
        "wt2t": np.ascontiguousarray(Wt2.T),
        "w2at": np.ascontiguousarray(Wt2.T) * w1[:, None],
        "w2bt": np.ascontiguousarray(Wt2.T) * (-2.0 * w1 ** 2)[:, None],
        "w2ct": np.ascontiguousarray(Wt2.T) * (6.0 * w1 ** 3)[:, None],
        "wt3": Wt3, "bt3c": bt3[:, None],
        "e0": blk(We2T), "e0m2": blk(-2.0 * We2T),
        "eq": blk(We2T * q[:, None]), "eqm2": blk(-2.0 * We2T * q[:, None]),
        "eq6": blk(6.0 * We2T * q[:, None]), "ep": blk(We2T * p[:, None]),
        "epm2": blk(-2.0 * We2T * p[:, None]),
        "be1b2": np.concatenate([be1, be1])[:, None],
        "be2b2": np.concatenate([be2, be2])[:, None],
        "lmat": lmat, "v6": v6,
    }
    smalls = {k: np.ascontiguousarray(val, np.float32) for k, val in smalls.items()}

    in_maps = []
    for c in range(NCORES):
        blk_w = Wb[:, c * KSH:(c + 1) * KSH]
        tr = blk_w.T.reshape(NCHUNK, JT, 128, 128).transpose(0, 2, 1, 3)
        tr = tr.reshape(NCHUNK, 128, JT * 128)
        hi = tr.astype(ml_dtypes.bfloat16)
        lo = (tr - hi.astype(np.float32)).astype(ml_dtypes.bfloat16)
        wsh = np.ascontiguousarray(np.stack([hi, lo], axis=1))
        ash = a[c * KSH:(c + 1) * KSH].reshape(KSH // 128, 128).T
        ahi = ash.astype(ml_dtypes.bfloat16)
        alo = (ash - ahi.astype(np.float32)).astype(ml_dtypes.bfloat16)
        a2 = np.ascontiguousarray(np.stack([ahi, alo], axis=2))
        xs = x[c * NPTS:(c + 1) * NPTS]
        x2 = np.ascontiguousarray(np.stack([xs, np.zeros_like(xs)]))
        im = {"w": wsh, "a2": a2, "x2": x2}
        im.update(smalls)
        in_maps.append(im)

    global _last_in_maps
    _last_in_maps = in_maps
    nc = _get_nc(MMDT_NAME)
    res = bass_utils.run_bass_kernel_spmd(nc, in_maps, core_ids=list(range(NCORES)))
    outs = []
    for c in range(NCORES):
        o = res.results[c]["out"]          # [2, NPTS//2]
        outs.append(o.reshape(-1))
    return np.concatenate(outs).astype(np.float32)
